# revision 31
# baseline (speedup 1.0000x reference)
"""nn_CustomAttention on 8 Trainium2 NeuronCores — flash-pipelined v2.

Full (unsharded) inputs in, full output out. Data-parallel over batch (2) x
tensor-parallel over heads (16 -> 4 per core).

Key structural ideas vs the phase-separated baseline:
  - "Flash" pipeline over key-strips: the QKV projection for strip w+1 runs
    on the tensor engine interleaved with the attention (exp-heavy, scalar
    engine) of key-strip w, so the ACT work hides under matmuls instead of
    serializing after the whole projection phase.
  - AV partial sums accumulate in SBUF (DVE adds from a small PSUM scratch)
    so all 8 (strip, head-pair) accumulators can be live at once; PSUM holds
    only scores (2x2 banks), AV scratch (2x1 banks) and one projection
    accumulator pair (2 banks).
  - Softmax denominator comes from a ones-column appended to V (row 64 of
    the AV accumulation); normalization is partition_broadcast of the
    denominator row, then reciprocal_approx_fast on [64,1024] (the baseline
    did reciprocal on [1,512] = single-lane DVE, 3.3us each).
  - Output projection partials and their 4-rank ReduceScatter run per strip
    in bf16 (1 MB instead of 2 MB f32 per strip), launched as each strip's
    attention completes so only the last strip's RS sits in the tail.
  - Projection matmuls run in bf16 (x and W pre-rounded on host); scores
    run in f32r on q,k stored f32r; AV runs bf16. All matmul dtypes are
    full-rate; bf16 halves DMA and SBUF so x stays resident per strip.
"""
import numpy as np

import concourse.bass as bass
import concourse.mybir as mybir
import concourse.tile as tile
from concourse import bacc, bass_utils
from concourse.alu_op_type import AluOpType

B, N, C, H, HD = 2, 2048, 1024, 16, 64
HPC = 4          # heads per core
TP = 4           # tensor-parallel group size
NCORES = 8
SW = 512         # strip width (queries and keys)
NSTRIPS = N // SW
NJC = N // 128   # key chunks of 128
SCALE = HD ** -0.5
F32 = mybir.dt.float32
F32R = mybir.dt.float32r
BF16 = mybir.dt.bfloat16
ExpF = mybir.ActivationFunctionType.Exp

_CACHE = {}
LAST_EXEC_TIME_NS = None


def _ensure_ntff_hook():
    """Register the axon NTFF profiling hook if the image's antenv lacks
    antenv.axon_hooks (needed only for trace=True timing runs)."""
    try:
        import antenv
        import importlib
        try:
            importlib.import_module("antenv.axon_hooks")
            return
        except ImportError:
            pass
        import sys
        import types
        mod = types.ModuleType("antenv.axon_hooks")
        mod._hook = None

        def set_axon_ntff_profile_hook(h):
            mod._hook = h

        def get_axon_ntff_profile_hook():
            return mod._hook

        mod.set_axon_ntff_profile_hook = set_axon_ntff_profile_hook
        mod.get_axon_ntff_profile_hook = get_axon_ntff_profile_hook
        sys.modules["antenv.axon_hooks"] = mod
        antenv.axon_hooks = mod
        from trn_agent_boot.trn_boot import _ntff_profile_via_ctypes
        hook = _ntff_profile_via_ctypes("/opt/axon/libaxon_pjrt.so")
        if hook is not None:
            set_axon_ntff_profile_hook(hook)
    except Exception:
        pass


def build_nc():
    nc = bacc.Bacc("TRN2", target_bir_lowering=False, debug=False,
                   num_devices=NCORES)
    xq = nc.dram_tensor("xq", [8, 128, N], BF16, kind="ExternalInput").ap()
    xk = nc.dram_tensor("xk", [8, 128, N], BF16, kind="ExternalInput").ap()
    xv = nc.dram_tensor("xv", [8, 128, N], BF16, kind="ExternalInput").ap()
    wqkv = nc.dram_tensor("wqkv", [24, 128, 768], BF16,
                          kind="ExternalInput").ap()
    wproj = nc.dram_tensor("wproj", [2, 128, C], BF16,
                           kind="ExternalInput").ap()
    bias = nc.dram_tensor("bias", [1, C], F32, kind="ExternalInput").ap()
    # y rows: [s*128,(s+1)*128) = this rank's 128-row chunk of strip s
    y = nc.dram_tensor("y", [N // TP, C], F32, kind="ExternalOutput").ap()
    xsrc = [xq, xk, xv]

    with tile.TileContext(nc) as tc:
        with tc.tile_pool(name="singles", bufs=1) as singles, \
             tc.tile_pool(name="xsa", bufs=2) as xsa, \
             tc.tile_pool(name="xsb", bufs=2) as xsb, \
             tc.tile_pool(name="ep", bufs=3) as ep, \
             tc.tile_pool(name="denp", bufs=1) as denp, \
             tc.tile_pool(name="oTp", bufs=2) as oTp, \
             tc.tile_pool(name="ytbp", bufs=2) as ytbp, \
             tc.tile_pool(name="ytp", bufs=1) as ytp, \
             tc.tile_pool(name="ps", bufs=1, space="PSUM") as ps, \
             tc.tile_pool(name="dram", bufs=1, space="DRAM") as dram:

            w_tiles = [singles.tile([128, 768], BF16, name=f"w{c}",
                                    tag=f"w{c}") for c in range(24)]
            for c in range(24):
                nc.sync.dma_start(w_tiles[c][:], wqkv[c])
            wp_tiles = [singles.tile([128, C], BF16, name=f"wp{i}",
                                     tag=f"wp{i}") for i in range(2)]
            for i in range(2):
                nc.sync.dma_start(wp_tiles[i][:], wproj[i])
            bias_sb = singles.tile([1, C], F32, name="bias_sb")
            nc.sync.dma_start(bias_sb[:], bias)
            bias_bc = singles.tile([128, C], F32, name="bias_bc")
            nc.gpsimd.partition_broadcast(bias_bc[:], bias_sb[:])
            ones65 = singles.tile([65, 64], F32, name="ones65")
            nc.vector.memset(ones65[:], 1.0)

            # q,k feature-major: fc 0,1 = q head-pairs; fc 2,3 = k head-pairs
            qk_sb = singles.tile([128, 4, N], BF16, name="qk_sb")
            # v key-major + ones column per head
            v_sb = singles.tile([128, NJC, HPC, 65], BF16, name="v_sb")
            ones1 = singles.tile([128, 1], F32, name="ones1")
            nc.vector.memset(ones1[:], 1.0)
            nc.vector.tensor_copy(
                v_sb[:, :, :, 64],
                ones1[:, :, None].to_broadcast([128, NJC, HPC]))
            # AV accumulators: rows 0..63 numerators, row 64 denominator;
            # cols [par*512,(par+1)*512) = head 2p+par over this strip's 512 q
            po_sb = [[singles.tile([65, 1024], F32, name=f"po{s}_{p}",
                                   tag=f"po{s}_{p}") for p in range(2)]
                     for s in range(NSTRIPS)]

            cc_in = [dram.tile([SW, C], BF16, name=f"cc_in{s}")
                     for s in range(NSTRIPS)]
            cc_out = [dram.tile([SW // TP, C], BF16, name=f"cc_out{s}")
                      for s in range(NSTRIPS)]

            # ---------------- emission helpers ----------------
            def load_xs(s):
                """DMA x (concat-feature chunks) for strip s into xsa/xsb."""
                a = xsa.tile([128, 12, SW], BF16, tag="xsa", name="xsa")
                b = xsb.tile([128, 12, SW], BF16, tag="xsb", name="xsb")
                for c in range(24):
                    dst = a if c < 12 else b
                    nc.sync.dma_start(
                        dst[:, c % 12, :],
                        xsrc[c // 8][c % 8, :, s * SW:(s + 1) * SW])
                return (a, b)

            def xchunk(xs, c):
                return xs[0][:, c, :] if c < 12 else xs[1][:, c - 12, :]

            def prod_tasks(s, xs):
                """Generator of (emit_mm_fns, drain_fn) for phase-A of strip
                s: k groups, then v, then q (consumers of k/v unlock
                earliest), each 24 accumulating MMs."""
                for i in (2, 3, None, 0, 1):   # fc 2,3 k; None -> v; 0,1 q
                    if i is None:
                        yield from v_tasks(s, xs)
                        continue
                    pa = ps.tile([128, SW], F32, tag="pa", name="pa", bufs=1)

                    def mk(c, i=i, pa=pa):
                        nc.tensor.matmul(
                            pa[:], w_tiles[c][:, i * 128:(i + 1) * 128],
                            xchunk(xs, c), start=(c == 0), stop=(c == 23))

                    def drain(i=i, pa=pa, s=s):
                        nc.vector.tensor_copy(
                            qk_sb[:, i, s * SW:(s + 1) * SW], pa[:])
                    yield [lambda c=c, mk=mk: mk(c) for c in range(24)], drain

            def v_tasks(s, xs):
                for ncn in range(4):    # v key-major chunks
                    pa = ps.tile([128, SW], F32, tag="pa", name="pa",
                                 bufs=1)[:, 0:256]

                    def mkv(c, ncn=ncn, pa=pa):
                        nc.tensor.matmul(
                            pa[:], xchunk(xs, c)[:, ncn * 128:(ncn + 1) * 128],
                            w_tiles[c][:, 512:768],
                            start=(c == 0), stop=(c == 23))

                    def drainv(ncn=ncn, pa=pa, s=s):
                        nc.vector.tensor_copy(
                            v_sb[:, s * 4 + ncn, :, 0:64],
                            pa[:].rearrange("p (h d) -> p h d", h=HPC))
                    yield [lambda c=c, mkv=mkv: mkv(c) for c in range(24)], \
                        drainv

            class Filler:
                """Flattens production tasks into a stream of small emit
                steps so they interleave with attention units."""

                def __init__(self, tasks):
                    self.steps = []
                    for mms, drain in tasks:
                        self.steps.extend(mms)
                        self.steps.append(drain)
                    self.i = 0

                def emit(self, k):
                    while k > 0 and self.i < len(self.steps):
                        self.steps[self.i]()
                        self.i += 1
                        k -= 1

                def flush(self):
                    self.emit(len(self.steps))

            pending_av = []
            cur_av = [None]

            def emit_pending_av():
                for fn in pending_av:
                    fn()
                del pending_av[:]

            def unit(s, t, p, j):
                """scores+exp for (strip s, key-chunk jc=t*4+j, pair p);
                AV+accumulate deferred via pending_av (1-unit lag)."""
                jc = t * 4 + j
                sc = ps.tile([128, 1024], F32, tag="sc", name="sc", bufs=2)
                for par in range(2):
                    hp = par * 64
                    nc.tensor.matmul(
                        sc[:, par * SW:(par + 1) * SW],
                        qk_sb[hp:hp + 64, 2 + p, jc * 128:(jc + 1) * 128],
                        qk_sb[hp:hp + 64, p, s * SW:(s + 1) * SW],
                        start=True, stop=True)
                et = ep.tile([128, 1024], BF16, tag="e", name="et")
                nc.scalar.activation(out=et[:], in_=sc[:], func=ExpF)

                def do_av(s=s, t=t, p=p, j=j, jc=jc, et=et):
                    if j == 0:
                        cur_av[0] = [ps.tile([65, SW], F32, tag="av",
                                             name="av", bufs=3)
                                     for _ in range(2)]
                    for par in range(2):
                        nc.tensor.matmul(
                            cur_av[0][par][:],
                            v_sb[:, jc, 2 * p + par, :],
                            et[:, par * SW:(par + 1) * SW],
                            start=(j == 0), stop=(j == 3))
                    if j == 3:
                        po = po_sb[s][p]
                        for par in range(2):
                            dst = po[:, par * SW:(par + 1) * SW]
                            if t == 0:
                                nc.vector.tensor_copy(dst, cur_av[0][par][:])
                            else:
                                nc.vector.tensor_add(dst, dst,
                                                     cur_av[0][par][:])
                pending_av.append(do_av)

            def cell(s, t, filler, per_unit_fill):
                for p in range(2):
                    for j in range(4):
                        unit(s, t, p, j)
                        filler.emit(per_unit_fill)
                        emit_pending_av_one()

            def emit_pending_av_one():
                if len(pending_av) > 2:
                    pending_av.pop(0)()

            def norm_proj_rs(s):
                """Normalize strip s, project partials (this core's 256
                features), add (rank-0-only) bias, bf16 ReduceScatter."""
                # ot: feature-major [128 = par*64+d, co = pair, n]
                ot = oTp.tile([128, 2, SW], BF16, tag="oT", name="ot")
                for p in range(2):
                    # broadcast den row (partition 64) to 64 partitions with
                    # a K=1 ones matmul — keeps the gpsimd queue (which
                    # blocks on collective completion) out of the norm path
                    den_ps = ps.tile([128, 1024], F32, tag="sc",
                                     name="den_ps", bufs=2)[0:64, :]
                    for mh in range(2):
                        nc.tensor.matmul(
                            den_ps[:, mh * SW:(mh + 1) * SW],
                            ones65[64:65, :],
                            po_sb[s][p][64:65, mh * SW:(mh + 1) * SW],
                            start=True, stop=True)
                    rec = denp.tile([64, 1024], F32, tag="rec", name="rec")
                    nc.vector.reciprocal_approx_fast(rec[:], den_ps[:])
                    for par in range(2):
                        nc.vector.tensor_mul(
                            ot[par * 64:(par + 1) * 64, p, :],
                            po_sb[s][p][0:64, par * SW:(par + 1) * SW],
                            rec[:, par * SW:(par + 1) * SW])
                for nch in range(4):
                    ytb = ytbp.tile([128, C], BF16, tag="ytb", name="ytb")
                    for mh in range(2):
                        pp = ps.tile([128, SW], F32, tag="pa", name="pp",
                                     bufs=1)
                        for co in range(2):
                            nc.tensor.matmul(
                                pp[:],
                                ot[:, co, nch * 128:(nch + 1) * 128],
                                wp_tiles[co][:, mh * SW:(mh + 1) * SW],
                                start=(co == 0), stop=(co == 1))
                        nc.vector.tensor_add(
                            ytb[:, mh * SW:(mh + 1) * SW], pp[:],
                            bias_bc[:, mh * SW:(mh + 1) * SW])
                    nc.sync.dma_start(
                        cc_in[s][nch * 128:(nch + 1) * 128, :], ytb[:])
                nc.gpsimd.collective_compute(
                    "ReduceScatter", AluOpType.add,
                    replica_groups=[[0, 1, 2, 3], [4, 5, 6, 7]],
                    ins=[cc_in[s][:].opt()],
                    outs=[cc_out[s][:].opt()])

            def finish_y(s):
                yb = ytbp.tile([128, C], BF16, tag="yb", name="yb")
                nc.sync.dma_start(yb[:], cc_out[s][:])
                yt = ytp.tile([128, C], F32, tag="yt", name="yt")
                nc.vector.tensor_copy(yt[:], yb[:])
                nc.sync.dma_start(y[s * 128:(s + 1) * 128, :], yt[:])

            # ---------------- schedule ----------------
            # prologue: load strip 0's x and produce its q,k,v densely
            xs = load_xs(0)
            f0 = Filler(prod_tasks(0, xs))
            f0.flush()

            xs_next = load_xs(1)
            for w in range(NSTRIPS):
                if w < NSTRIPS - 1:
                    filler = Filler(prod_tasks(w + 1, xs_next))
                else:
                    filler = Filler([])
                # cells ready this window: new strip w catches up on old
                # keys, then all strips consume key-strip w
                cells = [(w, t) for t in range(w)] + \
                        [(s, w) for s in range(w + 1)]
                done_after = {}
                if w == NSTRIPS - 2:
                    # strips 0..2 can consume key-strip 3 as soon as this
                    # window's filler produces k(3), v(3) (ordered first)
                    cells += [(0, 3), (1, 3), (2, 3)]
                    done_after = {(0, 3): 0, (1, 3): 1, (2, 3): 2}
                if w == NSTRIPS - 1:
                    cells = [(3, 0), (3, 1), (3, 2), (3, 3)]
                    done_after = {(3, 3): 3}
                nun = len(cells) * 8
                per_unit = (len(filler.steps) + nun - 1) // max(nun, 1)
                for ct in cells:
                    cell(ct[0], ct[1], filler, per_unit)
                    if ct in done_after:
                        s_done = done_after[ct]
                        emit_pending_av()
                        norm_proj_rs(s_done)
                        if s_done > 0:
                            finish_y(s_done - 1)
                filler.flush()
                emit_pending_av()
                if w < NSTRIPS - 2:
                    xs_next = load_xs(w + 2)
            finish_y(3)
    nc.compile()
    return nc


def make_in_maps(q, k, v, W_qkv, W_proj, b_proj):
    bf = mybir.dt.np(BF16)
    in_maps = []
    for core in range(NCORES):
        b, r = divmod(core, TP)
        lo, hi = r * HPC * HD, (r + 1) * HPC * HD    # this core's 256 features
        wq = W_qkv[lo:hi, :] * np.float32(SCALE)
        wk = W_qkv[C + lo:C + hi, :]
        wv = W_qkv[2 * C + lo:2 * C + hi, :]
        wsel = np.concatenate([wq, wk, wv], axis=0)        # [768, 3072]
        wqkvT = np.ascontiguousarray(wsel.T)               # [3072, 768]
        wprojT = np.ascontiguousarray(W_proj[:, lo:hi].T)  # [256, 1024]
        bias = b_proj if r == 0 else np.zeros_like(b_proj)
        in_maps.append({
            "xq": np.ascontiguousarray(q[b].T).reshape(8, 128, N).astype(bf),
            "xk": np.ascontiguousarray(k[b].T).reshape(8, 128, N).astype(bf),
            "xv": np.ascontiguousarray(v[b].T).reshape(8, 128, N).astype(bf),
            "wqkv": wqkvT.reshape(24, 128, 768).astype(bf),
            "wproj": wprojT.reshape(2, 128, C).astype(bf),
            "bias": np.ascontiguousarray(bias[None, :], dtype=np.float32),
        })
    return in_maps


def kernel(q, k, v, W_qkv, W_proj, b_proj, trace=False):
    global LAST_EXEC_TIME_NS
    q = np.asarray(q, dtype=np.float32)
    k = np.asarray(k, dtype=np.float32)
    v = np.asarray(v, dtype=np.float32)
    W_qkv = np.asarray(W_qkv, dtype=np.float32)
    W_proj = np.asarray(W_proj, dtype=np.float32)
    b_proj = np.asarray(b_proj, dtype=np.float32)

    if trace:
        _ensure_ntff_hook()
    if "nc" not in _CACHE:
        _CACHE["nc"] = build_nc()
    nc = _CACHE["nc"]
    in_maps = make_in_maps(q, k, v, W_qkv, W_proj, b_proj)
    res = bass_utils.run_bass_kernel_spmd(
        nc, in_maps, core_ids=list(range(NCORES)), trace=trace)
    LAST_EXEC_TIME_NS = res.exec_time_ns
    _CACHE["trace"] = getattr(res, "instructions_and_trace", None)

    out = np.empty((B, N, C), dtype=np.float32)
    Q = SW // TP   # 128 rows per (rank, strip)
    for core in range(NCORES):
        b, r = divmod(core, TP)
        ys = res.results[core]["y"]
        for s in range(NSTRIPS):
            out[b, s * SW + r * Q:s * SW + (r + 1) * Q, :] = ys[s * Q:(s + 1) * Q]
    return out


# revision 32
# speedup vs baseline: 1.0261x; 1.0261x over previous
"""nn_CustomAttention on 8 Trainium2 NeuronCores — flash-pipelined v2.

Full (unsharded) inputs in, full output out. Data-parallel over batch (2) x
tensor-parallel over heads (16 -> 4 per core).

Key structural ideas vs the phase-separated baseline:
  - "Flash" pipeline over key-strips: the QKV projection for strip w+1 runs
    on the tensor engine interleaved with the attention (exp-heavy, scalar
    engine) of key-strip w, so the ACT work hides under matmuls instead of
    serializing after the whole projection phase.
  - AV partial sums accumulate in SBUF (DVE adds from a small PSUM scratch)
    so all 8 (strip, head-pair) accumulators can be live at once; PSUM holds
    only scores (2x2 banks), AV scratch (2x1 banks) and one projection
    accumulator pair (2 banks).
  - Softmax denominator comes from a ones-column appended to V (row 64 of
    the AV accumulation); normalization is partition_broadcast of the
    denominator row, then reciprocal_approx_fast on [64,1024] (the baseline
    did reciprocal on [1,512] = single-lane DVE, 3.3us each).
  - Output projection partials and their 4-rank ReduceScatter run per strip
    in bf16 (1 MB instead of 2 MB f32 per strip), launched as each strip's
    attention completes so only the last strip's RS sits in the tail.
  - Projection matmuls run in bf16 (x and W pre-rounded on host); scores
    run in f32r on q,k stored f32r; AV runs bf16. All matmul dtypes are
    full-rate; bf16 halves DMA and SBUF so x stays resident per strip.
"""
import numpy as np

import concourse.bass as bass
import concourse.mybir as mybir
import concourse.tile as tile
from concourse import bacc, bass_utils
from concourse.alu_op_type import AluOpType

B, N, C, H, HD = 2, 2048, 1024, 16, 64
HPC = 4          # heads per core
TP = 4           # tensor-parallel group size
NCORES = 8
SW = 512         # strip width (queries and keys)
NSTRIPS = N // SW
NJC = N // 128   # key chunks of 128
SCALE = HD ** -0.5
F32 = mybir.dt.float32
F32R = mybir.dt.float32r
BF16 = mybir.dt.bfloat16
ExpF = mybir.ActivationFunctionType.Exp

_CACHE = {}
LAST_EXEC_TIME_NS = None


def _ensure_ntff_hook():
    """Register the axon NTFF profiling hook if the image's antenv lacks
    antenv.axon_hooks (needed only for trace=True timing runs)."""
    try:
        import antenv
        import importlib
        try:
            importlib.import_module("antenv.axon_hooks")
            return
        except ImportError:
            pass
        import sys
        import types
        mod = types.ModuleType("antenv.axon_hooks")
        mod._hook = None

        def set_axon_ntff_profile_hook(h):
            mod._hook = h

        def get_axon_ntff_profile_hook():
            return mod._hook

        mod.set_axon_ntff_profile_hook = set_axon_ntff_profile_hook
        mod.get_axon_ntff_profile_hook = get_axon_ntff_profile_hook
        sys.modules["antenv.axon_hooks"] = mod
        antenv.axon_hooks = mod
        from trn_agent_boot.trn_boot import _ntff_profile_via_ctypes
        hook = _ntff_profile_via_ctypes("/opt/axon/libaxon_pjrt.so")
        if hook is not None:
            set_axon_ntff_profile_hook(hook)
    except Exception:
        pass


def build_nc():
    nc = bacc.Bacc("TRN2", target_bir_lowering=False, debug=False,
                   num_devices=NCORES)
    xq = nc.dram_tensor("xq", [8, 128, N], BF16, kind="ExternalInput").ap()
    xk = nc.dram_tensor("xk", [8, 128, N], BF16, kind="ExternalInput").ap()
    xv = nc.dram_tensor("xv", [8, 128, N], BF16, kind="ExternalInput").ap()
    wqkv = nc.dram_tensor("wqkv", [24, 128, 768], BF16,
                          kind="ExternalInput").ap()
    wproj = nc.dram_tensor("wproj", [2, 128, C], BF16,
                           kind="ExternalInput").ap()
    bias = nc.dram_tensor("bias", [1, C], F32, kind="ExternalInput").ap()
    # y rows: [s*128,(s+1)*128) = this rank's 128-row chunk of strip s
    y = nc.dram_tensor("y", [N // TP, C], F32, kind="ExternalOutput").ap()
    xsrc = [xq, xk, xv]

    with tile.TileContext(nc) as tc:
        with tc.tile_pool(name="singles", bufs=1) as singles, \
             tc.tile_pool(name="xsa", bufs=2) as xsa, \
             tc.tile_pool(name="xsb", bufs=2) as xsb, \
             tc.tile_pool(name="ep", bufs=2) as ep, \
             tc.tile_pool(name="denp", bufs=1) as denp, \
             tc.tile_pool(name="oTp", bufs=2) as oTp, \
             tc.tile_pool(name="ytbp", bufs=2) as ytbp, \
             tc.tile_pool(name="ytp", bufs=1) as ytp, \
             tc.tile_pool(name="ps", bufs=1, space="PSUM") as ps, \
             tc.tile_pool(name="dram", bufs=1, space="DRAM") as dram:

            w_tiles = [singles.tile([128, 768], BF16, name=f"w{c}",
                                    tag=f"w{c}") for c in range(24)]
            for c in range(24):
                nc.sync.dma_start(w_tiles[c][:], wqkv[c])
            wp_tiles = [singles.tile([128, C], BF16, name=f"wp{i}",
                                     tag=f"wp{i}") for i in range(2)]
            for i in range(2):
                nc.sync.dma_start(wp_tiles[i][:], wproj[i])
            bias_sb = singles.tile([1, C], F32, name="bias_sb")
            nc.sync.dma_start(bias_sb[:], bias)
            bias_bc = singles.tile([128, C], F32, name="bias_bc")
            nc.gpsimd.partition_broadcast(bias_bc[:], bias_sb[:])
            ones65 = singles.tile([65, 64], F32, name="ones65")
            nc.vector.memset(ones65[:], 1.0)

            # q,k feature-major: fc 0,1 = q head-pairs; fc 2,3 = k head-pairs
            qk_sb = singles.tile([128, 4, N], BF16, name="qk_sb")
            # v key-major + ones column per head
            v_sb = singles.tile([128, NJC, HPC, 65], BF16, name="v_sb")
            ones1 = singles.tile([128, 1], F32, name="ones1")
            nc.vector.memset(ones1[:], 1.0)
            nc.vector.tensor_copy(
                v_sb[:, :, :, 64],
                ones1[:, :, None].to_broadcast([128, NJC, HPC]))
            # AV accumulators: rows 0..63 numerators, row 64 denominator;
            # cols [par*512,(par+1)*512) = head 2p+par over this strip's 512 q
            po_sb = [[singles.tile([65, 1024], F32, name=f"po{s}_{p}",
                                   tag=f"po{s}_{p}") for p in range(2)]
                     for s in range(NSTRIPS)]

            cc_in = [dram.tile([SW, C], BF16, name=f"cc_in{s}")
                     for s in range(NSTRIPS)]
            cc_out = [dram.tile([SW // TP, C], BF16, name=f"cc_out{s}")
                      for s in range(NSTRIPS)]

            # ---------------- emission helpers ----------------
            def load_xs(s):
                """DMA x (concat-feature chunks) for strip s into xsa/xsb."""
                a = xsa.tile([128, 12, SW], BF16, tag="xsa", name="xsa")
                b = xsb.tile([128, 12, SW], BF16, tag="xsb", name="xsb")
                for c in range(24):
                    dst = a if c < 12 else b
                    nc.sync.dma_start(
                        dst[:, c % 12, :],
                        xsrc[c // 8][c % 8, :, s * SW:(s + 1) * SW])
                return (a, b)

            def xchunk(xs, c):
                return xs[0][:, c, :] if c < 12 else xs[1][:, c - 12, :]

            def prod_tasks(s, xs):
                """Generator of (emit_mm_fns, drain_fn) for phase-A of strip
                s: k groups, then v, then q (consumers of k/v unlock
                earliest), each 24 accumulating MMs."""
                for i in (2, 3, None, 0, 1):   # fc 2,3 k; None -> v; 0,1 q
                    if i is None:
                        yield from v_tasks(s, xs)
                        continue
                    pa = ps.tile([128, SW], F32, tag="pa", name="pa", bufs=2)

                    def mk(c, i=i, pa=pa):
                        nc.tensor.matmul(
                            pa[:], w_tiles[c][:, i * 128:(i + 1) * 128],
                            xchunk(xs, c), start=(c == 0), stop=(c == 23))

                    def drain(i=i, pa=pa, s=s):
                        nc.scalar.copy(
                            qk_sb[:, i, s * SW:(s + 1) * SW], pa[:])
                    yield [lambda c=c, mk=mk: mk(c) for c in range(24)], drain

            def v_tasks(s, xs):
                for ncn in range(4):    # v key-major chunks
                    pa = ps.tile([128, SW], F32, tag="pa", name="pa",
                                 bufs=2)[:, 0:256]

                    def mkv(c, ncn=ncn, pa=pa):
                        nc.tensor.matmul(
                            pa[:], xchunk(xs, c)[:, ncn * 128:(ncn + 1) * 128],
                            w_tiles[c][:, 512:768],
                            start=(c == 0), stop=(c == 23))

                    def drainv(ncn=ncn, pa=pa, s=s):
                        nc.scalar.copy(
                            v_sb[:, s * 4 + ncn, :, 0:64],
                            pa[:].rearrange("p (h d) -> p h d", h=HPC))
                    yield [lambda c=c, mkv=mkv: mkv(c) for c in range(24)], \
                        drainv

            class Filler:
                """Flattens production tasks into a stream of small emit
                steps so they interleave with attention units."""

                def __init__(self, tasks):
                    self.steps = []
                    for mms, drain in tasks:
                        self.steps.extend(mms)
                        self.steps.append(drain)
                    self.i = 0

                def emit(self, k):
                    while k > 0 and self.i < len(self.steps):
                        self.steps[self.i]()
                        self.i += 1
                        k -= 1

                def flush(self):
                    self.emit(len(self.steps))

            pending_av = []
            cur_av = [None]

            def emit_pending_av():
                for fn in pending_av:
                    fn()
                del pending_av[:]

            def unit(s, t, p, j):
                """scores+exp for (strip s, key-chunk jc=t*4+j, pair p);
                AV+accumulate deferred via pending_av (1-unit lag)."""
                jc = t * 4 + j
                sc = ps.tile([128, 1024], F32, tag="sc", name="sc", bufs=2)
                for par in range(2):
                    hp = par * 64
                    nc.tensor.matmul(
                        sc[:, par * SW:(par + 1) * SW],
                        qk_sb[hp:hp + 64, 2 + p, jc * 128:(jc + 1) * 128],
                        qk_sb[hp:hp + 64, p, s * SW:(s + 1) * SW],
                        start=True, stop=True)
                et = ep.tile([128, 1024], BF16, tag="e", name="et")
                nc.scalar.activation(out=et[:], in_=sc[:], func=ExpF)

                def do_av(s=s, t=t, p=p, j=j, jc=jc, et=et):
                    if j == 0:
                        cur_av[0] = [ps.tile([65, SW], F32, tag="av",
                                             name="av", bufs=2)
                                     for _ in range(2)]
                    for par in range(2):
                        nc.tensor.matmul(
                            cur_av[0][par][:],
                            v_sb[:, jc, 2 * p + par, :],
                            et[:, par * SW:(par + 1) * SW],
                            start=(j == 0), stop=(j == 3))
                    if j == 3:
                        po = po_sb[s][p]
                        for par in range(2):
                            dst = po[:, par * SW:(par + 1) * SW]
                            if t == 0:
                                nc.vector.tensor_copy(dst, cur_av[0][par][:])
                            else:
                                nc.vector.tensor_add(dst, dst,
                                                     cur_av[0][par][:])
                pending_av.append(do_av)

            def cell(s, t, filler, per_unit_fill):
                for p in range(2):
                    for j in range(4):
                        unit(s, t, p, j)
                        filler.emit(per_unit_fill)
                        emit_pending_av_one()

            def emit_pending_av_one():
                if len(pending_av) > 1:
                    pending_av.pop(0)()

            def norm_proj_rs(s):
                """Normalize strip s, project partials (this core's 256
                features), add (rank-0-only) bias, bf16 ReduceScatter."""
                # ot: feature-major [128 = par*64+d, co = pair, n]
                ot = oTp.tile([128, 2, SW], BF16, tag="oT", name="ot")
                for p in range(2):
                    # broadcast den row (partition 64) to 64 partitions with
                    # a K=1 ones matmul — keeps the gpsimd queue (which
                    # blocks on collective completion) out of the norm path
                    den_ps = ps.tile([128, 1024], F32, tag="sc",
                                     name="den_ps", bufs=2)[0:64, :]
                    for mh in range(2):
                        nc.tensor.matmul(
                            den_ps[:, mh * SW:(mh + 1) * SW],
                            ones65[64:65, :],
                            po_sb[s][p][64:65, mh * SW:(mh + 1) * SW],
                            start=True, stop=True)
                    rec = denp.tile([64, 1024], F32, tag="rec", name="rec")
                    nc.vector.reciprocal_approx_fast(rec[:], den_ps[:])
                    for par in range(2):
                        nc.vector.tensor_mul(
                            ot[par * 64:(par + 1) * 64, p, :],
                            po_sb[s][p][0:64, par * SW:(par + 1) * SW],
                            rec[:, par * SW:(par + 1) * SW])
                for nch in range(4):
                    ytb = ytbp.tile([128, C], BF16, tag="ytb", name="ytb")
                    for mh in range(2):
                        pp = ps.tile([128, SW], F32, tag="pa", name="pp",
                                     bufs=2)
                        for co in range(2):
                            nc.tensor.matmul(
                                pp[:],
                                ot[:, co, nch * 128:(nch + 1) * 128],
                                wp_tiles[co][:, mh * SW:(mh + 1) * SW],
                                start=(co == 0), stop=(co == 1))
                        nc.vector.tensor_add(
                            ytb[:, mh * SW:(mh + 1) * SW], pp[:],
                            bias_bc[:, mh * SW:(mh + 1) * SW])
                    nc.sync.dma_start(
                        cc_in[s][nch * 128:(nch + 1) * 128, :], ytb[:])
                nc.gpsimd.collective_compute(
                    "ReduceScatter", AluOpType.add,
                    replica_groups=[[0, 1, 2, 3], [4, 5, 6, 7]],
                    ins=[cc_in[s][:].opt()],
                    outs=[cc_out[s][:].opt()])

            def finish_y(s):
                yb = ytbp.tile([128, C], BF16, tag="yb", name="yb")
                nc.sync.dma_start(yb[:], cc_out[s][:])
                yt = ytp.tile([128, C], F32, tag="yt", name="yt")
                nc.vector.tensor_copy(yt[:], yb[:])
                nc.sync.dma_start(y[s * 128:(s + 1) * 128, :], yt[:])

            # ---------------- schedule ----------------
            # prologue: load strip 0's x and produce its q,k,v densely
            xs = load_xs(0)
            f0 = Filler(prod_tasks(0, xs))
            f0.flush()

            xs_next = load_xs(1)
            for w in range(NSTRIPS):
                if w < NSTRIPS - 1:
                    filler = Filler(prod_tasks(w + 1, xs_next))
                else:
                    filler = Filler([])
                # cells ready this window: new strip w catches up on old
                # keys, then all strips consume key-strip w
                cells = [(w, t) for t in range(w)] + \
                        [(s, w) for s in range(w + 1)]
                done_after = {}
                if w == NSTRIPS - 2:
                    # strips 0..2 can consume key-strip 3 as soon as this
                    # window's filler produces k(3), v(3) (ordered first)
                    cells += [(0, 3), (1, 3), (2, 3)]
                    done_after = {(0, 3): 0, (1, 3): 1, (2, 3): 2}
                if w == NSTRIPS - 1:
                    cells = [(3, 0), (3, 1), (3, 2), (3, 3)]
                    done_after = {(3, 3): 3}
                nun = len(cells) * 8
                per_unit = (len(filler.steps) + nun - 1) // max(nun, 1)
                for ct in cells:
                    cell(ct[0], ct[1], filler, per_unit)
                    if ct in done_after:
                        s_done = done_after[ct]
                        emit_pending_av()
                        norm_proj_rs(s_done)
                        if s_done > 0:
                            finish_y(s_done - 1)
                filler.flush()
                emit_pending_av()
                if w < NSTRIPS - 2:
                    xs_next = load_xs(w + 2)
            finish_y(3)
    nc.compile()
    return nc


def make_in_maps(q, k, v, W_qkv, W_proj, b_proj):
    bf = mybir.dt.np(BF16)
    in_maps = []
    for core in range(NCORES):
        b, r = divmod(core, TP)
        lo, hi = r * HPC * HD, (r + 1) * HPC * HD    # this core's 256 features
        wq = W_qkv[lo:hi, :] * np.float32(SCALE)
        wk = W_qkv[C + lo:C + hi, :]
        wv = W_qkv[2 * C + lo:2 * C + hi, :]
        wsel = np.concatenate([wq, wk, wv], axis=0)        # [768, 3072]
        wqkvT = np.ascontiguousarray(wsel.T)               # [3072, 768]
        wprojT = np.ascontiguousarray(W_proj[:, lo:hi].T)  # [256, 1024]
        bias = b_proj if r == 0 else np.zeros_like(b_proj)
        in_maps.append({
            "xq": np.ascontiguousarray(q[b].T).reshape(8, 128, N).astype(bf),
            "xk": np.ascontiguousarray(k[b].T).reshape(8, 128, N).astype(bf),
            "xv": np.ascontiguousarray(v[b].T).reshape(8, 128, N).astype(bf),
            "wqkv": wqkvT.reshape(24, 128, 768).astype(bf),
            "wproj": wprojT.reshape(2, 128, C).astype(bf),
            "bias": np.ascontiguousarray(bias[None, :], dtype=np.float32),
        })
    return in_maps


def kernel(q, k, v, W_qkv, W_proj, b_proj, trace=False):
    global LAST_EXEC_TIME_NS
    q = np.asarray(q, dtype=np.float32)
    k = np.asarray(k, dtype=np.float32)
    v = np.asarray(v, dtype=np.float32)
    W_qkv = np.asarray(W_qkv, dtype=np.float32)
    W_proj = np.asarray(W_proj, dtype=np.float32)
    b_proj = np.asarray(b_proj, dtype=np.float32)

    if trace:
        _ensure_ntff_hook()
    if "nc" not in _CACHE:
        _CACHE["nc"] = build_nc()
    nc = _CACHE["nc"]
    in_maps = make_in_maps(q, k, v, W_qkv, W_proj, b_proj)
    res = bass_utils.run_bass_kernel_spmd(
        nc, in_maps, core_ids=list(range(NCORES)), trace=trace)
    LAST_EXEC_TIME_NS = res.exec_time_ns
    _CACHE["trace"] = getattr(res, "instructions_and_trace", None)

    out = np.empty((B, N, C), dtype=np.float32)
    Q = SW // TP   # 128 rows per (rank, strip)
    for core in range(NCORES):
        b, r = divmod(core, TP)
        ys = res.results[core]["y"]
        for s in range(NSTRIPS):
            out[b, s * SW + r * Q:s * SW + (r + 1) * Q, :] = ys[s * Q:(s + 1) * Q]
    return out


# revision 37
# speedup vs baseline: 1.0513x; 1.0246x over previous
"""nn_CustomAttention on 8 Trainium2 NeuronCores — flash-pipelined v2.

Full (unsharded) inputs in, full output out. Data-parallel over batch (2) x
tensor-parallel over heads (16 -> 4 per core).

Key structural ideas vs the phase-separated baseline:
  - "Flash" pipeline over key-strips: the QKV projection for strip w+1 runs
    on the tensor engine interleaved with the attention (exp-heavy, scalar
    engine) of key-strip w, so the ACT work hides under matmuls instead of
    serializing after the whole projection phase.
  - AV partial sums accumulate in SBUF (DVE adds from a small PSUM scratch)
    so all 8 (strip, head-pair) accumulators can be live at once; PSUM holds
    only scores (2x2 banks), AV scratch (2x1 banks) and one projection
    accumulator pair (2 banks).
  - Softmax denominator comes from a ones-column appended to V (row 64 of
    the AV accumulation); normalization is partition_broadcast of the
    denominator row, then reciprocal_approx_fast on [64,1024] (the baseline
    did reciprocal on [1,512] = single-lane DVE, 3.3us each).
  - Output projection partials and their 4-rank ReduceScatter run per strip
    in bf16 (1 MB instead of 2 MB f32 per strip), launched as each strip's
    attention completes so only the last strip's RS sits in the tail.
  - Projection matmuls run in bf16 (x and W pre-rounded on host); scores
    run in f32r on q,k stored f32r; AV runs bf16. All matmul dtypes are
    full-rate; bf16 halves DMA and SBUF so x stays resident per strip.
"""
import numpy as np

import concourse.bass as bass
import concourse.mybir as mybir
import concourse.tile as tile
from concourse import bacc, bass_utils
from concourse.alu_op_type import AluOpType

B, N, C, H, HD = 2, 2048, 1024, 16, 64
HPC = 4          # heads per core
TP = 4           # tensor-parallel group size
NCORES = 8
SW = 512         # strip width (queries and keys)
NSTRIPS = N // SW
NJC = N // 128   # key chunks of 128
SCALE = HD ** -0.5
F32 = mybir.dt.float32
F32R = mybir.dt.float32r
BF16 = mybir.dt.bfloat16
ExpF = mybir.ActivationFunctionType.Exp

_CACHE = {}
LAST_EXEC_TIME_NS = None


def _ensure_ntff_hook():
    """Register the axon NTFF profiling hook if the image's antenv lacks
    antenv.axon_hooks (needed only for trace=True timing runs)."""
    try:
        import antenv
        import importlib
        try:
            importlib.import_module("antenv.axon_hooks")
            return
        except ImportError:
            pass
        import sys
        import types
        mod = types.ModuleType("antenv.axon_hooks")
        mod._hook = None

        def set_axon_ntff_profile_hook(h):
            mod._hook = h

        def get_axon_ntff_profile_hook():
            return mod._hook

        mod.set_axon_ntff_profile_hook = set_axon_ntff_profile_hook
        mod.get_axon_ntff_profile_hook = get_axon_ntff_profile_hook
        sys.modules["antenv.axon_hooks"] = mod
        antenv.axon_hooks = mod
        from trn_agent_boot.trn_boot import _ntff_profile_via_ctypes
        hook = _ntff_profile_via_ctypes("/opt/axon/libaxon_pjrt.so")
        if hook is not None:
            set_axon_ntff_profile_hook(hook)
    except Exception:
        pass


def build_nc():
    nc = bacc.Bacc("TRN2", target_bir_lowering=False, debug=False,
                   num_devices=NCORES)
    xq = nc.dram_tensor("xq", [8, 128, N], BF16, kind="ExternalInput").ap()
    xk = nc.dram_tensor("xk", [8, 128, N], BF16, kind="ExternalInput").ap()
    xv = nc.dram_tensor("xv", [8, 128, N], BF16, kind="ExternalInput").ap()
    wqkv = nc.dram_tensor("wqkv", [24, 128, 768], BF16,
                          kind="ExternalInput").ap()
    wproj = nc.dram_tensor("wproj", [2, 128, C], BF16,
                           kind="ExternalInput").ap()
    bias = nc.dram_tensor("bias", [1, C], F32, kind="ExternalInput").ap()
    # y rows: [s*128,(s+1)*128) = this rank's 128-row chunk of strip s
    # (bf16: filled by plain DRAM->DRAM DMA from the RS output; host upcasts)
    y = nc.dram_tensor("y", [N // TP, C], BF16, kind="ExternalOutput").ap()
    xsrc = [xq, xk, xv]

    with tile.TileContext(nc) as tc:
        with tc.tile_pool(name="singles", bufs=1) as singles, \
             tc.tile_pool(name="xsa", bufs=2) as xsa, \
             tc.tile_pool(name="xsb", bufs=2) as xsb, \
             tc.tile_pool(name="ep", bufs=3) as ep, \
             tc.tile_pool(name="denp", bufs=1) as denp, \
             tc.tile_pool(name="oTp", bufs=2) as oTp, \
             tc.tile_pool(name="ytbp", bufs=2) as ytbp, \
             tc.tile_pool(name="ytp", bufs=1) as ytp, \
             tc.tile_pool(name="ps", bufs=1, space="PSUM") as ps, \
             tc.tile_pool(name="dram", bufs=1, space="DRAM") as dram:

            w_tiles = [singles.tile([128, 768], BF16, name=f"w{c}",
                                    tag=f"w{c}") for c in range(24)]
            for c in range(24):
                nc.sync.dma_start(w_tiles[c][:], wqkv[c])
            wp_tiles = [singles.tile([128, C], BF16, name=f"wp{i}",
                                     tag=f"wp{i}") for i in range(2)]
            for i in range(2):
                nc.sync.dma_start(wp_tiles[i][:], wproj[i])
            bias_sb = singles.tile([1, C], F32, name="bias_sb")
            nc.sync.dma_start(bias_sb[:], bias)
            bias_bc = singles.tile([128, C], F32, name="bias_bc")
            nc.gpsimd.partition_broadcast(bias_bc[:], bias_sb[:])
            ones65 = singles.tile([65, 64], F32, name="ones65")
            nc.vector.memset(ones65[:], 1.0)

            # q,k feature-major: fc 0,1 = q head-pairs; fc 2,3 = k head-pairs
            qk_sb = singles.tile([128, 4, N], BF16, name="qk_sb")
            # v key-major + ones column per head
            v_sb = singles.tile([128, NJC, HPC, 65], BF16, name="v_sb")
            ones1 = singles.tile([128, 1], F32, name="ones1")
            nc.vector.memset(ones1[:], 1.0)
            nc.vector.tensor_copy(
                v_sb[:, :, :, 64],
                ones1[:, :, None].to_broadcast([128, NJC, HPC]))
            # AV accumulators: rows 0..63 numerators, row 64 denominator;
            # cols [par*512,(par+1)*512) = head 2p+par over this strip's 512 q
            po_sb = [[singles.tile([65, 1024], F32, name=f"po{s}_{p}",
                                   tag=f"po{s}_{p}") for p in range(2)]
                     for s in range(NSTRIPS)]

            cc_in = [dram.tile([SW, C], BF16, name=f"cc_in{s}")
                     for s in range(NSTRIPS)]
            cc_out = [dram.tile([SW // TP, C], BF16, name=f"cc_out{s}")
                      for s in range(NSTRIPS)]

            # ---------------- emission helpers ----------------
            def load_xs(s):
                """DMA x (concat-feature chunks) for strip s into xsa/xsb."""
                a = xsa.tile([128, 12, SW], BF16, tag="xsa", name="xsa")
                b = xsb.tile([128, 12, SW], BF16, tag="xsb", name="xsb")
                for c in range(24):
                    dst = a if c < 12 else b
                    nc.sync.dma_start(
                        dst[:, c % 12, :],
                        xsrc[c // 8][c % 8, :, s * SW:(s + 1) * SW])
                return (a, b)

            def xchunk(xs, c):
                return xs[0][:, c, :] if c < 12 else xs[1][:, c - 12, :]

            def prod_tasks(s, xs):
                """Generator of (emit_mm_fns, drain_fn) for phase-A of strip
                s: k groups, then v, then q (consumers of k/v unlock
                earliest), each 24 accumulating MMs."""
                for i in (2, 3, None, 0, 1):   # fc 2,3 k; None -> v; 0,1 q
                    if i is None:
                        yield from v_tasks(s, xs)
                        continue
                    pa = ps.tile([128, SW], F32, tag="pa", name="pa", bufs=2)

                    def mk(c, i=i, pa=pa):
                        nc.tensor.matmul(
                            pa[:], w_tiles[c][:, i * 128:(i + 1) * 128],
                            xchunk(xs, c), start=(c == 0), stop=(c == 23))

                    def drain(i=i, pa=pa, s=s):
                        nc.scalar.copy(
                            qk_sb[:, i, s * SW:(s + 1) * SW], pa[:])
                    yield [lambda c=c, mk=mk: mk(c) for c in range(24)], drain

            def v_tasks(s, xs):
                for ncn in range(4):    # v key-major chunks
                    pa = ps.tile([128, SW], F32, tag="pa", name="pa",
                                 bufs=2)[:, 0:256]

                    def mkv(c, ncn=ncn, pa=pa):
                        nc.tensor.matmul(
                            pa[:], xchunk(xs, c)[:, ncn * 128:(ncn + 1) * 128],
                            w_tiles[c][:, 512:768],
                            start=(c == 0), stop=(c == 23))

                    def drainv(ncn=ncn, pa=pa, s=s):
                        nc.scalar.copy(
                            v_sb[:, s * 4 + ncn, :, 0:64],
                            pa[:].rearrange("p (h d) -> p h d", h=HPC))
                    yield [lambda c=c, mkv=mkv: mkv(c) for c in range(24)], \
                        drainv

            class Filler:
                """Flattens production tasks into a stream of small emit
                steps so they interleave with attention units."""

                def __init__(self, tasks):
                    self.steps = []
                    for mms, drain in tasks:
                        self.steps.extend(mms)
                        self.steps.append(drain)
                    self.i = 0

                def emit(self, k):
                    while k > 0 and self.i < len(self.steps):
                        self.steps[self.i]()
                        self.i += 1
                        k -= 1

                def flush(self):
                    self.emit(len(self.steps))

            pending_av = []
            cur_av = [None]

            def emit_pending_av():
                for fn in pending_av:
                    fn()
                del pending_av[:]

            def unit(s, t, p, j):
                """scores+exp for (strip s, key-chunk jc=t*4+j, pair p);
                AV+accumulate deferred via pending_av (1-unit lag)."""
                jc = t * 4 + j
                sc = ps.tile([128, 1024], F32, tag="sc", name="sc", bufs=2)
                for par in range(2):
                    hp = par * 64
                    nc.tensor.matmul(
                        sc[:, par * SW:(par + 1) * SW],
                        qk_sb[hp:hp + 64, 2 + p, jc * 128:(jc + 1) * 128],
                        qk_sb[hp:hp + 64, p, s * SW:(s + 1) * SW],
                        start=True, stop=True)
                et = ep.tile([128, 1024], BF16, tag="e", name="et")
                nc.scalar.activation(out=et[:], in_=sc[:], func=ExpF)

                def do_av(s=s, t=t, p=p, j=j, jc=jc, et=et):
                    if j == 0:
                        cur_av[0] = [ps.tile([65, SW], F32, tag="av",
                                             name="av", bufs=2)
                                     for _ in range(2)]
                    for par in range(2):
                        nc.tensor.matmul(
                            cur_av[0][par][:],
                            v_sb[:, jc, 2 * p + par, :],
                            et[:, par * SW:(par + 1) * SW],
                            start=(j == 0), stop=(j == 3))
                    if j == 3:
                        po = po_sb[s][p]
                        for par in range(2):
                            dst = po[:, par * SW:(par + 1) * SW]
                            if t == 0:
                                nc.vector.tensor_copy(dst, cur_av[0][par][:])
                            else:
                                nc.vector.tensor_add(dst, dst,
                                                     cur_av[0][par][:])
                pending_av.append(do_av)

            def cell(s, t, filler, per_unit_fill):
                for p in range(2):
                    for j in range(4):
                        unit(s, t, p, j)
                        filler.emit(per_unit_fill)
                        emit_pending_av_one()

            def emit_pending_av_one():
                if len(pending_av) > 1:
                    pending_av.pop(0)()

            def norm_proj_rs(s):
                """Normalize strip s, project partials (this core's 256
                features), add (rank-0-only) bias, bf16 ReduceScatter."""
                # ot: feature-major [128 = par*64+d, co = pair, n]
                ot = oTp.tile([128, 2, SW], BF16, tag="oT", name="ot")
                for p in range(2):
                    # broadcast den row (partition 64) to 64 partitions with
                    # a K=1 ones matmul — keeps the gpsimd queue (which
                    # blocks on collective completion) out of the norm path
                    den_ps = ps.tile([128, 1024], F32, tag="sc",
                                     name="den_ps", bufs=2)[0:64, :]
                    for mh in range(2):
                        nc.tensor.matmul(
                            den_ps[:, mh * SW:(mh + 1) * SW],
                            ones65[64:65, :],
                            po_sb[s][p][64:65, mh * SW:(mh + 1) * SW],
                            start=True, stop=True)
                    rec = denp.tile([64, 1024], F32, tag="rec", name="rec")
                    nc.vector.reciprocal_approx_fast(rec[:], den_ps[:])
                    for par in range(2):
                        nc.vector.tensor_mul(
                            ot[par * 64:(par + 1) * 64, p, :],
                            po_sb[s][p][0:64, par * SW:(par + 1) * SW],
                            rec[:, par * SW:(par + 1) * SW])
                for nch in range(4):
                    ytb = ytbp.tile([128, C], BF16, tag="ytb", name="ytb")
                    for mh in range(2):
                        pp = ps.tile([128, SW], F32, tag="pa", name="pp",
                                     bufs=2)
                        for co in range(2):
                            nc.tensor.matmul(
                                pp[:],
                                ot[:, co, nch * 128:(nch + 1) * 128],
                                wp_tiles[co][:, mh * SW:(mh + 1) * SW],
                                start=(co == 0), stop=(co == 1))
                        nc.vector.tensor_add(
                            ytb[:, mh * SW:(mh + 1) * SW], pp[:],
                            bias_bc[:, mh * SW:(mh + 1) * SW])
                    nc.sync.dma_start(
                        cc_in[s][nch * 128:(nch + 1) * 128, :], ytb[:])
                nc.gpsimd.collective_compute(
                    "ReduceScatter", AluOpType.add,
                    replica_groups=[[0, 1, 2, 3], [4, 5, 6, 7]],
                    ins=[cc_in[s][:].opt()],
                    outs=[cc_out[s][:].opt()])

            def finish_y(s):
                nc.sync.dma_start(y[s * 128:(s + 1) * 128, :], cc_out[s][:])

            # ---------------- schedule ----------------
            # prologue: load strip 0's x and produce its q,k,v chunk-outer
            # (4 accumulators live per pass) so matmuls start as soon as
            # each x/w chunk's DMA lands instead of after the whole load
            xs = load_xs(0)
            specs = [("k", 2, "sc"), ("k", 3, "sc"), ("v", 0, "pa"),
                     ("v", 1, "pa"), ("v", 2, "sc"), ("v", 3, "sc"),
                     ("q", 0, "pa"), ("q", 1, "pa")]
            for half in (specs[:4], specs[4:]):
                tiles = {}
                for kind, i, tag in half:
                    if tag == "sc":
                        tiles[(kind, i)] = ps.tile(
                            [128, 1024], F32, tag="sc", name="plg",
                            bufs=2)[:, 0:SW]
                    else:
                        tiles[(kind, i)] = ps.tile(
                            [128, SW], F32, tag="pa", name="pa", bufs=2)
                for c in range(24):
                    for kind, i, tag in half:
                        t = tiles[(kind, i)]
                        if kind == "v":
                            nc.tensor.matmul(
                                t[:, 0:256],
                                xchunk(xs, c)[:, i * 128:(i + 1) * 128],
                                w_tiles[c][:, 512:768],
                                start=(c == 0), stop=(c == 23))
                        else:
                            nc.tensor.matmul(
                                t[:], w_tiles[c][:, i * 128:(i + 1) * 128],
                                xchunk(xs, c),
                                start=(c == 0), stop=(c == 23))
                for kind, i, tag in half:
                    t = tiles[(kind, i)]
                    if kind == "v":
                        nc.scalar.copy(
                            v_sb[:, i, :, 0:64],
                            t[:, 0:256].rearrange("p (h d) -> p h d", h=HPC))
                    else:
                        nc.scalar.copy(qk_sb[:, i, 0:SW], t[:])

            xs_next = load_xs(1)
            for w in range(NSTRIPS):
                if w < NSTRIPS - 1:
                    filler = Filler(prod_tasks(w + 1, xs_next))
                else:
                    filler = Filler([])
                # cells ready this window: new strip w catches up on old
                # keys, then all strips consume key-strip w
                cells = [(w, t) for t in range(w)] + \
                        [(s, w) for s in range(w + 1)]
                done_after = {}
                if w == NSTRIPS - 2:
                    # strips 0..2 can consume key-strip 3 as soon as this
                    # window's filler produces k(3), v(3) (ordered first)
                    cells += [(0, 3), (1, 3), (2, 3)]
                    done_after = {(0, 3): 0, (1, 3): 1, (2, 3): 2}
                if w == NSTRIPS - 1:
                    cells = [(3, 0), (3, 1), (3, 2), (3, 3)]
                    done_after = {(3, 3): 3}
                nun = len(cells) * 8
                per_unit = (len(filler.steps) + nun - 1) // max(nun, 1)
                for ct in cells:
                    cell(ct[0], ct[1], filler, per_unit)
                    if ct in done_after:
                        s_done = done_after[ct]
                        emit_pending_av()
                        norm_proj_rs(s_done)
                        if s_done > 0:
                            finish_y(s_done - 1)
                filler.flush()
                emit_pending_av()
                if w < NSTRIPS - 2:
                    xs_next = load_xs(w + 2)
            finish_y(3)
    nc.compile()
    return nc


def make_in_maps(q, k, v, W_qkv, W_proj, b_proj):
    bf = mybir.dt.np(BF16)
    in_maps = []
    for core in range(NCORES):
        b, r = divmod(core, TP)
        lo, hi = r * HPC * HD, (r + 1) * HPC * HD    # this core's 256 features
        wq = W_qkv[lo:hi, :] * np.float32(SCALE)
        wk = W_qkv[C + lo:C + hi, :]
        wv = W_qkv[2 * C + lo:2 * C + hi, :]
        wsel = np.concatenate([wq, wk, wv], axis=0)        # [768, 3072]
        wqkvT = np.ascontiguousarray(wsel.T)               # [3072, 768]
        wprojT = np.ascontiguousarray(W_proj[:, lo:hi].T)  # [256, 1024]
        bias = b_proj if r == 0 else np.zeros_like(b_proj)
        in_maps.append({
            "xq": np.ascontiguousarray(q[b].T).reshape(8, 128, N).astype(bf),
            "xk": np.ascontiguousarray(k[b].T).reshape(8, 128, N).astype(bf),
            "xv": np.ascontiguousarray(v[b].T).reshape(8, 128, N).astype(bf),
            "wqkv": wqkvT.reshape(24, 128, 768).astype(bf),
            "wproj": wprojT.reshape(2, 128, C).astype(bf),
            "bias": np.ascontiguousarray(bias[None, :], dtype=np.float32),
        })
    return in_maps


def kernel(q, k, v, W_qkv, W_proj, b_proj, trace=False):
    global LAST_EXEC_TIME_NS
    q = np.asarray(q, dtype=np.float32)
    k = np.asarray(k, dtype=np.float32)
    v = np.asarray(v, dtype=np.float32)
    W_qkv = np.asarray(W_qkv, dtype=np.float32)
    W_proj = np.asarray(W_proj, dtype=np.float32)
    b_proj = np.asarray(b_proj, dtype=np.float32)

    if trace:
        _ensure_ntff_hook()
    if "nc" not in _CACHE:
        _CACHE["nc"] = build_nc()
    nc = _CACHE["nc"]
    in_maps = make_in_maps(q, k, v, W_qkv, W_proj, b_proj)
    res = bass_utils.run_bass_kernel_spmd(
        nc, in_maps, core_ids=list(range(NCORES)), trace=trace)
    LAST_EXEC_TIME_NS = res.exec_time_ns
    _CACHE["trace"] = getattr(res, "instructions_and_trace", None)

    out = np.empty((B, N, C), dtype=np.float32)
    Q = SW // TP   # 128 rows per (rank, strip)
    for core in range(NCORES):
        b, r = divmod(core, TP)
        ys = np.asarray(res.results[core]["y"], dtype=np.float32)
        for s in range(NSTRIPS):
            out[b, s * SW + r * Q:s * SW + (r + 1) * Q, :] = ys[s * Q:(s + 1) * Q]
    return out


# revision 39
# speedup vs baseline: 1.0693x; 1.0171x over previous
"""nn_CustomAttention on 8 Trainium2 NeuronCores — flash-pipelined v2.

Full (unsharded) inputs in, full output out. Data-parallel over batch (2) x
tensor-parallel over heads (16 -> 4 per core).

Key structural ideas vs the phase-separated baseline:
  - "Flash" pipeline over key-strips: the QKV projection for strip w+1 runs
    on the tensor engine interleaved with the attention (exp-heavy, scalar
    engine) of key-strip w, so the ACT work hides under matmuls instead of
    serializing after the whole projection phase.
  - AV partial sums accumulate in SBUF (DVE adds from a small PSUM scratch)
    so all 8 (strip, head-pair) accumulators can be live at once; PSUM holds
    only scores (2x2 banks), AV scratch (2x1 banks) and one projection
    accumulator pair (2 banks).
  - Softmax denominator comes from a ones-column appended to V (row 64 of
    the AV accumulation); normalization is partition_broadcast of the
    denominator row, then reciprocal_approx_fast on [64,1024] (the baseline
    did reciprocal on [1,512] = single-lane DVE, 3.3us each).
  - Output projection partials and their 4-rank ReduceScatter run per strip
    in bf16 (1 MB instead of 2 MB f32 per strip), launched as each strip's
    attention completes so only the last strip's RS sits in the tail.
  - Projection matmuls run in bf16 (x and W pre-rounded on host); scores
    run in f32r on q,k stored f32r; AV runs bf16. All matmul dtypes are
    full-rate; bf16 halves DMA and SBUF so x stays resident per strip.
"""
import numpy as np

import concourse.bass as bass
import concourse.mybir as mybir
import concourse.tile as tile
from concourse import bacc, bass_utils
from concourse.alu_op_type import AluOpType

B, N, C, H, HD = 2, 2048, 1024, 16, 64
HPC = 4          # heads per core
TP = 4           # tensor-parallel group size
NCORES = 8
SW = 512         # strip width (queries and keys)
NSTRIPS = N // SW
NJC = N // 128   # key chunks of 128
SCALE = HD ** -0.5
F32 = mybir.dt.float32
F32R = mybir.dt.float32r
BF16 = mybir.dt.bfloat16
ExpF = mybir.ActivationFunctionType.Exp

_CACHE = {}
LAST_EXEC_TIME_NS = None


def _ensure_ntff_hook():
    """Register the axon NTFF profiling hook if the image's antenv lacks
    antenv.axon_hooks (needed only for trace=True timing runs)."""
    try:
        import antenv
        import importlib
        try:
            importlib.import_module("antenv.axon_hooks")
            return
        except ImportError:
            pass
        import sys
        import types
        mod = types.ModuleType("antenv.axon_hooks")
        mod._hook = None

        def set_axon_ntff_profile_hook(h):
            mod._hook = h

        def get_axon_ntff_profile_hook():
            return mod._hook

        mod.set_axon_ntff_profile_hook = set_axon_ntff_profile_hook
        mod.get_axon_ntff_profile_hook = get_axon_ntff_profile_hook
        sys.modules["antenv.axon_hooks"] = mod
        antenv.axon_hooks = mod
        from trn_agent_boot.trn_boot import _ntff_profile_via_ctypes
        hook = _ntff_profile_via_ctypes("/opt/axon/libaxon_pjrt.so")
        if hook is not None:
            set_axon_ntff_profile_hook(hook)
    except Exception:
        pass


def build_nc():
    nc = bacc.Bacc("TRN2", target_bir_lowering=False, debug=False,
                   num_devices=NCORES)
    xq = nc.dram_tensor("xq", [8, 128, N], BF16, kind="ExternalInput").ap()
    xk = nc.dram_tensor("xk", [8, 128, N], BF16, kind="ExternalInput").ap()
    xv = nc.dram_tensor("xv", [8, 128, N], BF16, kind="ExternalInput").ap()
    wqkv = nc.dram_tensor("wqkv", [24, 128, 768], BF16,
                          kind="ExternalInput").ap()
    wproj = nc.dram_tensor("wproj", [2, 128, C], BF16,
                           kind="ExternalInput").ap()
    bias = nc.dram_tensor("bias", [1, C], F32, kind="ExternalInput").ap()
    # y rows: [s*128,(s+1)*128) = this rank's 128-row chunk of strip s
    # (bf16: filled by plain DRAM->DRAM DMA from the RS output; host upcasts)
    y = nc.dram_tensor("y", [N // TP, C], BF16, kind="ExternalOutput").ap()
    xsrc = [xq, xk, xv]

    with tile.TileContext(nc) as tc:
        with tc.tile_pool(name="singles", bufs=1) as singles, \
             tc.tile_pool(name="xsa", bufs=2) as xsa, \
             tc.tile_pool(name="xsb", bufs=2) as xsb, \
             tc.tile_pool(name="ep", bufs=3) as ep, \
             tc.tile_pool(name="denp", bufs=1) as denp, \
             tc.tile_pool(name="oTp", bufs=2) as oTp, \
             tc.tile_pool(name="ytbp", bufs=2) as ytbp, \
             tc.tile_pool(name="ytp", bufs=1) as ytp, \
             tc.tile_pool(name="ps", bufs=1, space="PSUM") as ps, \
             tc.tile_pool(name="dram", bufs=1, space="DRAM") as dram:

            w_tiles = [singles.tile([128, 768], BF16, name=f"w{c}",
                                    tag=f"w{c}") for c in range(24)]
            # interleave strip-0 x chunks with the weight chunks so the
            # chunk-outer prologue's matmul for chunk c can start as soon
            # as its own pair of DMAs lands (not after all 24 w DMAs)
            xs0a = xsa.tile([128, 12, SW], BF16, tag="xsa", name="xsa")
            xs0b = xsb.tile([128, 12, SW], BF16, tag="xsb", name="xsb")
            for c in range(24):
                nc.sync.dma_start(
                    (xs0a if c < 12 else xs0b)[:, c % 12, :],
                    [xq, xk, xv][c // 8][c % 8, :, 0:SW])
                nc.sync.dma_start(w_tiles[c][:], wqkv[c])
            xs0 = (xs0a, xs0b)
            wp_tiles = [singles.tile([128, C], BF16, name=f"wp{i}",
                                     tag=f"wp{i}") for i in range(2)]
            for i in range(2):
                nc.sync.dma_start(wp_tiles[i][:], wproj[i])
            bias_sb = singles.tile([1, C], F32, name="bias_sb")
            nc.sync.dma_start(bias_sb[:], bias)
            bias_bc = singles.tile([128, C], F32, name="bias_bc")
            nc.gpsimd.partition_broadcast(bias_bc[:], bias_sb[:])
            ones65 = singles.tile([65, 64], F32, name="ones65")
            nc.vector.memset(ones65[:], 1.0)

            # q,k feature-major: fc 0,1 = q head-pairs; fc 2,3 = k head-pairs
            qk_sb = singles.tile([128, 4, N], BF16, name="qk_sb")
            # v key-major + ones column per head
            v_sb = singles.tile([128, NJC, HPC, 65], BF16, name="v_sb")
            ones1 = singles.tile([128, 1], F32, name="ones1")
            nc.vector.memset(ones1[:], 1.0)
            nc.vector.tensor_copy(
                v_sb[:, :, :, 64],
                ones1[:, :, None].to_broadcast([128, NJC, HPC]))
            # AV accumulators: rows 0..63 numerators, row 64 denominator;
            # cols [par*512,(par+1)*512) = head 2p+par over this strip's 512 q
            po_sb = [[singles.tile([65, 1024], F32, name=f"po{s}_{p}",
                                   tag=f"po{s}_{p}") for p in range(2)]
                     for s in range(NSTRIPS)]

            cc_in = [dram.tile([SW, C], BF16, name=f"cc_in{s}")
                     for s in range(NSTRIPS)]
            cc_out = [dram.tile([SW // TP, C], BF16, name=f"cc_out{s}")
                      for s in range(NSTRIPS)]

            # ---------------- emission helpers ----------------
            def load_xs(s):
                """DMA x (concat-feature chunks) for strip s into xsa/xsb."""
                a = xsa.tile([128, 12, SW], BF16, tag="xsa", name="xsa")
                b = xsb.tile([128, 12, SW], BF16, tag="xsb", name="xsb")
                for c in range(24):
                    dst = a if c < 12 else b
                    nc.sync.dma_start(
                        dst[:, c % 12, :],
                        xsrc[c // 8][c % 8, :, s * SW:(s + 1) * SW])
                return (a, b)

            def xchunk(xs, c):
                return xs[0][:, c, :] if c < 12 else xs[1][:, c - 12, :]

            def prod_tasks(s, xs):
                """Generator of (emit_mm_fns, drain_fn) for phase-A of strip
                s: k groups, then v, then q (consumers of k/v unlock
                earliest), each 24 accumulating MMs."""
                for i in (2, 3, None, 0, 1):   # fc 2,3 k; None -> v; 0,1 q
                    if i is None:
                        yield from v_tasks(s, xs)
                        continue
                    pa = ps.tile([128, SW], F32, tag="pa", name="pa", bufs=2)

                    def mk(c, i=i, pa=pa):
                        nc.tensor.matmul(
                            pa[:], w_tiles[c][:, i * 128:(i + 1) * 128],
                            xchunk(xs, c), start=(c == 0), stop=(c == 23))

                    def drain(i=i, pa=pa, s=s):
                        nc.scalar.copy(
                            qk_sb[:, i, s * SW:(s + 1) * SW], pa[:])
                    yield [lambda c=c, mk=mk: mk(c) for c in range(24)], drain

            def v_tasks(s, xs):
                for ncn in range(4):    # v key-major chunks
                    pa = ps.tile([128, SW], F32, tag="pa", name="pa",
                                 bufs=2)[:, 0:256]

                    def mkv(c, ncn=ncn, pa=pa):
                        nc.tensor.matmul(
                            pa[:], xchunk(xs, c)[:, ncn * 128:(ncn + 1) * 128],
                            w_tiles[c][:, 512:768],
                            start=(c == 0), stop=(c == 23))

                    def drainv(ncn=ncn, pa=pa, s=s):
                        nc.scalar.copy(
                            v_sb[:, s * 4 + ncn, :, 0:64],
                            pa[:].rearrange("p (h d) -> p h d", h=HPC))
                    yield [lambda c=c, mkv=mkv: mkv(c) for c in range(24)], \
                        drainv

            class Filler:
                """Flattens production tasks into a stream of small emit
                steps so they interleave with attention units."""

                def __init__(self, tasks):
                    self.steps = []
                    for mms, drain in tasks:
                        self.steps.extend(mms)
                        self.steps.append(drain)
                    self.i = 0

                def emit(self, k):
                    while k > 0 and self.i < len(self.steps):
                        self.steps[self.i]()
                        self.i += 1
                        k -= 1

                def flush(self):
                    self.emit(len(self.steps))

            pending_av = []
            cur_av = [None]

            def emit_pending_av():
                for fn in pending_av:
                    fn()
                del pending_av[:]

            def unit(s, t, p, j):
                """scores+exp for (strip s, key-chunk jc=t*4+j, pair p);
                AV+accumulate deferred via pending_av (1-unit lag)."""
                jc = t * 4 + j
                sc = ps.tile([128, 1024], F32, tag="sc", name="sc", bufs=2)
                for par in range(2):
                    hp = par * 64
                    nc.tensor.matmul(
                        sc[:, par * SW:(par + 1) * SW],
                        qk_sb[hp:hp + 64, 2 + p, jc * 128:(jc + 1) * 128],
                        qk_sb[hp:hp + 64, p, s * SW:(s + 1) * SW],
                        start=True, stop=True)
                et = ep.tile([128, 1024], BF16, tag="e", name="et")
                nc.scalar.activation(out=et[:], in_=sc[:], func=ExpF)

                def do_av(s=s, t=t, p=p, j=j, jc=jc, et=et):
                    if j == 0:
                        cur_av[0] = [ps.tile([65, SW], F32, tag="av",
                                             name="av", bufs=2)
                                     for _ in range(2)]
                    for par in range(2):
                        nc.tensor.matmul(
                            cur_av[0][par][:],
                            v_sb[:, jc, 2 * p + par, :],
                            et[:, par * SW:(par + 1) * SW],
                            start=(j == 0), stop=(j == 3))
                    if j == 3:
                        po = po_sb[s][p]
                        for par in range(2):
                            dst = po[:, par * SW:(par + 1) * SW]
                            if t == 0:
                                nc.vector.tensor_copy(dst, cur_av[0][par][:])
                            else:
                                nc.vector.tensor_add(dst, dst,
                                                     cur_av[0][par][:])
                pending_av.append(do_av)

            def cell(s, t, filler, per_unit_fill):
                for p in range(2):
                    for j in range(4):
                        unit(s, t, p, j)
                        filler.emit(per_unit_fill)
                        emit_pending_av_one()

            def emit_pending_av_one():
                if len(pending_av) > 1:
                    pending_av.pop(0)()

            def norm_proj_rs(s):
                """Normalize strip s, project partials (this core's 256
                features), add (rank-0-only) bias, bf16 ReduceScatter."""
                # ot: feature-major [128 = par*64+d, co = pair, n]
                ot = oTp.tile([128, 2, SW], BF16, tag="oT", name="ot")
                for p in range(2):
                    # broadcast den row (partition 64) to 64 partitions with
                    # a K=1 ones matmul — keeps the gpsimd queue (which
                    # blocks on collective completion) out of the norm path
                    den_ps = ps.tile([128, 1024], F32, tag="sc",
                                     name="den_ps", bufs=2)[0:64, :]
                    for mh in range(2):
                        nc.tensor.matmul(
                            den_ps[:, mh * SW:(mh + 1) * SW],
                            ones65[64:65, :],
                            po_sb[s][p][64:65, mh * SW:(mh + 1) * SW],
                            start=True, stop=True)
                    rec = denp.tile([64, 1024], F32, tag="rec", name="rec")
                    nc.vector.reciprocal_approx_fast(rec[:], den_ps[:])
                    for par in range(2):
                        nc.vector.tensor_mul(
                            ot[par * 64:(par + 1) * 64, p, :],
                            po_sb[s][p][0:64, par * SW:(par + 1) * SW],
                            rec[:, par * SW:(par + 1) * SW])
                for nch in range(4):
                    ytb = ytbp.tile([128, C], BF16, tag="ytb", name="ytb")
                    for mh in range(2):
                        pp = ps.tile([128, SW], F32, tag="pa", name="pp",
                                     bufs=2)
                        for co in range(2):
                            nc.tensor.matmul(
                                pp[:],
                                ot[:, co, nch * 128:(nch + 1) * 128],
                                wp_tiles[co][:, mh * SW:(mh + 1) * SW],
                                start=(co == 0), stop=(co == 1))
                        nc.vector.tensor_add(
                            ytb[:, mh * SW:(mh + 1) * SW], pp[:],
                            bias_bc[:, mh * SW:(mh + 1) * SW])
                    nc.sync.dma_start(
                        cc_in[s][nch * 128:(nch + 1) * 128, :], ytb[:])
                nc.gpsimd.collective_compute(
                    "ReduceScatter", AluOpType.add,
                    replica_groups=[[0, 1, 2, 3], [4, 5, 6, 7]],
                    ins=[cc_in[s][:].opt()],
                    outs=[cc_out[s][:].opt()])

            def finish_y(s):
                nc.sync.dma_start(y[s * 128:(s + 1) * 128, :], cc_out[s][:])

            # ---------------- schedule ----------------
            # prologue: produce strip 0's q,k,v chunk-outer (4 accumulators
            # live per pass) so matmuls start as soon as each x/w chunk's
            # DMA (interleaved above) lands instead of after the whole load
            xs = xs0
            specs = [("k", 2, "sc"), ("k", 3, "sc"), ("v", 0, "pa"),
                     ("v", 1, "pa"), ("v", 2, "sc"), ("v", 3, "sc"),
                     ("q", 0, "pa"), ("q", 1, "pa")]
            for half in (specs[:4], specs[4:]):
                tiles = {}
                for kind, i, tag in half:
                    if tag == "sc":
                        tiles[(kind, i)] = ps.tile(
                            [128, 1024], F32, tag="sc", name="plg",
                            bufs=2)[:, 0:SW]
                    else:
                        tiles[(kind, i)] = ps.tile(
                            [128, SW], F32, tag="pa", name="pa", bufs=2)
                for c in range(24):
                    for kind, i, tag in half:
                        t = tiles[(kind, i)]
                        if kind == "v":
                            nc.tensor.matmul(
                                t[:, 0:256],
                                xchunk(xs, c)[:, i * 128:(i + 1) * 128],
                                w_tiles[c][:, 512:768],
                                start=(c == 0), stop=(c == 23))
                        else:
                            nc.tensor.matmul(
                                t[:], w_tiles[c][:, i * 128:(i + 1) * 128],
                                xchunk(xs, c),
                                start=(c == 0), stop=(c == 23))
                for kind, i, tag in half:
                    t = tiles[(kind, i)]
                    if kind == "v":
                        nc.scalar.copy(
                            v_sb[:, i, :, 0:64],
                            t[:, 0:256].rearrange("p (h d) -> p h d", h=HPC))
                    else:
                        nc.scalar.copy(qk_sb[:, i, 0:SW], t[:])

            xs_next = load_xs(1)
            for w in range(NSTRIPS):
                if w < NSTRIPS - 1:
                    filler = Filler(prod_tasks(w + 1, xs_next))
                else:
                    filler = Filler([])
                # cells ready this window: new strip w catches up on old
                # keys, then all strips consume key-strip w
                cells = [(w, t) for t in range(w)] + \
                        [(s, w) for s in range(w + 1)]
                done_after = {}
                if w == NSTRIPS - 2:
                    # strips 0..2 can consume key-strip 3 as soon as this
                    # window's filler produces k(3), v(3) (ordered first)
                    cells += [(0, 3), (1, 3), (2, 3)]
                    done_after = {(0, 3): 0, (1, 3): 1, (2, 3): 2}
                if w == NSTRIPS - 1:
                    cells = [(3, 0), (3, 1), (3, 2), (3, 3)]
                    done_after = {(3, 3): 3}
                nun = len(cells) * 8
                per_unit = (len(filler.steps) + nun - 1) // max(nun, 1)
                for ct in cells:
                    cell(ct[0], ct[1], filler, per_unit)
                    if ct in done_after:
                        s_done = done_after[ct]
                        emit_pending_av()
                        norm_proj_rs(s_done)
                        if s_done > 0:
                            finish_y(s_done - 1)
                filler.flush()
                emit_pending_av()
                if w < NSTRIPS - 2:
                    xs_next = load_xs(w + 2)
            finish_y(3)
    nc.compile()
    return nc


def make_in_maps(q, k, v, W_qkv, W_proj, b_proj):
    bf = mybir.dt.np(BF16)
    in_maps = []
    for core in range(NCORES):
        b, r = divmod(core, TP)
        lo, hi = r * HPC * HD, (r + 1) * HPC * HD    # this core's 256 features
        wq = W_qkv[lo:hi, :] * np.float32(SCALE)
        wk = W_qkv[C + lo:C + hi, :]
        wv = W_qkv[2 * C + lo:2 * C + hi, :]
        wsel = np.concatenate([wq, wk, wv], axis=0)        # [768, 3072]
        wqkvT = np.ascontiguousarray(wsel.T)               # [3072, 768]
        wprojT = np.ascontiguousarray(W_proj[:, lo:hi].T)  # [256, 1024]
        bias = b_proj if r == 0 else np.zeros_like(b_proj)
        in_maps.append({
            "xq": np.ascontiguousarray(q[b].T).reshape(8, 128, N).astype(bf),
            "xk": np.ascontiguousarray(k[b].T).reshape(8, 128, N).astype(bf),
            "xv": np.ascontiguousarray(v[b].T).reshape(8, 128, N).astype(bf),
            "wqkv": wqkvT.reshape(24, 128, 768).astype(bf),
            "wproj": wprojT.reshape(2, 128, C).astype(bf),
            "bias": np.ascontiguousarray(bias[None, :], dtype=np.float32),
        })
    return in_maps


def kernel(q, k, v, W_qkv, W_proj, b_proj, trace=False):
    global LAST_EXEC_TIME_NS
    q = np.asarray(q, dtype=np.float32)
    k = np.asarray(k, dtype=np.float32)
    v = np.asarray(v, dtype=np.float32)
    W_qkv = np.asarray(W_qkv, dtype=np.float32)
    W_proj = np.asarray(W_proj, dtype=np.float32)
    b_proj = np.asarray(b_proj, dtype=np.float32)

    if trace:
        _ensure_ntff_hook()
    if "nc" not in _CACHE:
        _CACHE["nc"] = build_nc()
    nc = _CACHE["nc"]
    in_maps = make_in_maps(q, k, v, W_qkv, W_proj, b_proj)
    res = bass_utils.run_bass_kernel_spmd(
        nc, in_maps, core_ids=list(range(NCORES)), trace=trace)
    LAST_EXEC_TIME_NS = res.exec_time_ns
    _CACHE["trace"] = getattr(res, "instructions_and_trace", None)

    out = np.empty((B, N, C), dtype=np.float32)
    Q = SW // TP   # 128 rows per (rank, strip)
    for core in range(NCORES):
        b, r = divmod(core, TP)
        ys = np.asarray(res.results[core]["y"], dtype=np.float32)
        for s in range(NSTRIPS):
            out[b, s * SW + r * Q:s * SW + (r + 1) * Q, :] = ys[s * Q:(s + 1) * Q]
    return out
